# revision 5
# baseline (speedup 1.0000x reference)
"""Multi-head causal attention (B=2, S=2048, E=1024, H=16, D=64) on 8 TRN2
NeuronCores.

Sharding (data + tensor parallel, Megatron-style):
  core c -> batch b = c // 4, head group g = c % 4 (4 heads, e' = 256 cols).
  Wq/Wk/Wv column-sharded ([256, 1024] slices), Wo row-sharded
  ([1024, 256] slice); each core produces a partial output [2048, 1024]
  which the host sums per batch group (the Megatron all-reduce) and adds bo.

Per-core device kernel (matmul operands fp16, accumulate fp32 in PSUM):
  K^T = Wk_l x_k^T + bk  [256, 2048]   (e' on partitions -> heads x 64)
  Q^T = Wq_l x_q^T + bq  [256, 2048]
  V'  = [x_v Wv_l^T + bv | 1]  (ones col -> softmax denominator)
  attention in S^T orientation: per (q-tile 512, head-pair chunk), S^T
  tiles [128 k, 512 q] via PE with 2 heads packed in PE row groups
  0-63/64-127, exp on ACT (1/8 scale folded), multiplicative causal mask
  on the exp (diagonal tiles only), acc += V'^T @ P^T accumulated in PSUM
  [65, 512] whose row 64 is the softmax denominator; normalize via DVE
  reciprocal + GpSimd partition broadcast; O-projection from vals^T,
  emitted per q-tile. The k-loop interleaves both head-pair chunks and
  software-pipelines attnV one step behind exp so ACT stays saturated
  while PE works.
"""
import sys
import os

sys.path.insert(0, "/opt/trn_rl_repo")

import numpy as np
from contextlib import ExitStack

import concourse.bass as bass  # noqa: E402
import concourse.mybir as mybir  # noqa: E402
import concourse.tile as tile  # noqa: E402
from concourse import bacc, bass_utils  # noqa: E402

bass_utils.upload_artifacts = lambda d: f"local:{d}"

B, S, E, H, D = 2, 2048, 1024, 16, 64
NCORES = 8
EL = 256  # e' columns per core (4 heads)
F32 = mybir.dt.float32
F16 = mybir.dt.float16
AF = mybir.ActivationFunctionType
NP16 = np.float16

_CACHE = {}


def _build():
    nc = bacc.Bacc("TRN2", target_bir_lowering=False, debug=False)

    xq_d = nc.dram_tensor("xqT", [E, S], F16, kind="ExternalInput")
    xk_d = nc.dram_tensor("xkT", [E, S], F16, kind="ExternalInput")
    xv_d = nc.dram_tensor("xvT", [E, S], F16, kind="ExternalInput")
    wq_d = nc.dram_tensor("wqT", [E, EL], F16, kind="ExternalInput")
    wk_d = nc.dram_tensor("wkT", [E, EL], F16, kind="ExternalInput")
    wv_d = nc.dram_tensor("wvT", [E, EL], F16, kind="ExternalInput")
    wo_d = nc.dram_tensor("woT", [EL, E], F16, kind="ExternalInput")
    bq_d = nc.dram_tensor("bq", [EL], F32, kind="ExternalInput")
    bk_d = nc.dram_tensor("bk", [EL], F32, kind="ExternalInput")
    bv_d = nc.dram_tensor("bv", [EL], F32, kind="ExternalInput")
    vones_d = nc.dram_tensor("vones", [128, 16, 4, 1], F16, kind="ExternalInput")
    mask_d = nc.dram_tensor("masks", [4, 128, 512], F16, kind="ExternalInput")
    out_d = nc.dram_tensor("out", [S, E], F32, kind="ExternalOutput")

    with tile.TileContext(nc) as tc, ExitStack() as ctx:
        cpool = ctx.enter_context(tc.tile_pool(name="const", bufs=1))
        psp = ctx.enter_context(tc.tile_pool(name="psp", bufs=2, space="PSUM"))
        expp = ctx.enter_context(tc.tile_pool(name="expp", bufs=6))
        opool = ctx.enter_context(tc.tile_pool(name="op", bufs=4))
        smp = ctx.enter_context(tc.tile_pool(name="smp", bufs=4))

        # ---- constants + inputs, in consumption order (K, V, Q, then O) ----
        wk = cpool.tile([128, 8, EL], F16, tag="wk")
        nc.sync.dma_start(wk[:], wk_d.ap().rearrange("(k p) m -> p k m", p=128))
        bkt = cpool.tile([128, 2], F32, tag="bkt")
        nc.sync.dma_start(bkt[:], bk_d.ap().rearrange("(c p) -> p c", p=128))
        xk = cpool.tile([128, 8, S], F16, tag="xk")
        nc.sync.dma_start(xk[:], xk_d.ap().rearrange("(k p) m -> p k m", p=128))

        wv = cpool.tile([128, 8, EL], F16, tag="wv")
        nc.sync.dma_start(wv[:], wv_d.ap().rearrange("(k p) m -> p k m", p=128))
        bvr = cpool.tile([1, EL], F32, tag="bvr")
        nc.sync.dma_start(bvr[:], bv_d.ap().rearrange("(p m) -> p m", p=1))
        bvb = cpool.tile([128, EL], F32, tag="bvb")
        nc.gpsimd.partition_broadcast(bvb[:], bvr[:])
        xv = cpool.tile([128, 8, S], F16, tag="xv")
        nc.sync.dma_start(xv[:], xv_d.ap().rearrange("(k p) m -> p k m", p=128))

        wq = cpool.tile([128, 8, EL], F16, tag="wq")
        nc.sync.dma_start(wq[:], wq_d.ap().rearrange("(k p) m -> p k m", p=128))
        bqt = cpool.tile([128, 2], F32, tag="bqt")
        nc.sync.dma_start(bqt[:], bq_d.ap().rearrange("(c p) -> p c", p=128))
        xq = cpool.tile([128, 8, S], F16, tag="xq")
        nc.sync.dma_start(xq[:], xq_d.ap().rearrange("(k p) m -> p k m", p=128))

        mk = cpool.tile([128, 4, 512], F16, tag="mk")
        nc.sync.dma_start(mk[:], mask_d.ap().rearrange("k p m -> p k m"))
        wo = cpool.tile([128, 2, E], F16, tag="wo")
        nc.sync.dma_start(wo[:], wo_d.ap().rearrange("(c p) m -> p c m", p=128))

        KT = cpool.tile([128, 2, S], F16, tag="KT")
        QT = cpool.tile([128, 2, S], F16, tag="QT")
        VP = cpool.tile([128, 16, 4 * 66], F16, tag="VP")  # 66: 4B-aligned blocks
        valsT = cpool.tile([128, 2, S], F16, tag="valsT")

        # ones columns of V' (col 64 of each 66-block)
        nc.sync.dma_start(
            VP[:].rearrange("p k (h x) -> p k h x", h=4)[:, :, :, 64:65],
            vones_d.ap(),
        )

        # ---- transposed projections: out^T[e', t] = W x^T + b ----
        def proj_T(x_t, w_t, bias_t, out_t, pfx):
            for c in range(2):
                for tt in range(4):
                    ps = psp.tile([128, 512], F32, tag="lg", bufs=2,
                                  name=f"{pfx}ps{c}_{tt}")
                    for k in range(8):
                        nc.tensor.matmul(
                            ps[:],
                            lhsT=w_t[:, k, c * 128:(c + 1) * 128],
                            rhs=x_t[:, k, tt * 512:(tt + 1) * 512],
                            start=(k == 0), stop=(k == 7))
                    nc.vector.tensor_scalar_add(
                        out_t[:, c, tt * 512:(tt + 1) * 512],
                        ps[:], bias_t[:, c:c + 1])

        proj_T(xk, wk, bkt, KT, "k")

        # ---- V projection (natural layout) ----
        for t3 in range(16):
            ps = psp.tile([128, EL], F32, tag="lg", bufs=2, name=f"vps{t3}")
            for k in range(8):
                nc.tensor.matmul(
                    ps[:],
                    lhsT=xv[:, k, t3 * 128:(t3 + 1) * 128],
                    rhs=wv[:, k, :],
                    start=(k == 0), stop=(k == 7))
            nc.vector.tensor_add(
                VP[:, t3, :].rearrange("p (h x) -> p h x", h=4)[:, :, 0:64],
                ps[:].rearrange("p (h x) -> p h x", h=4),
                bvb[:].rearrange("p (h x) -> p h x", h=4))

        proj_T(xq, wq, bqt, QT, "q")

        # ---- attention + O-projection per q-tile ----
        for qt in range(4):
            nkt = 4 * qt + 4
            accs = {}
            for c in range(2):
                for hh in range(2):
                    accs[(c, hh)] = psp.tile([65, 512], F32, tag="acc",
                                             bufs=4, name=f"acc{qt}_{c}_{hh}")
            exs = {}

            def lg_exp(c, kt):
                lg = psp.tile([128, 2, 512], F32, tag="lg", bufs=2,
                              name=f"lg{qt}_{c}_{kt}")
                for hh in range(2):
                    nc.tensor.matmul(
                        lg[:, hh, :],
                        lhsT=KT[hh * 64:(hh + 1) * 64, c,
                                kt * 128:(kt + 1) * 128],
                        rhs=QT[hh * 64:(hh + 1) * 64, c,
                               qt * 512:(qt + 1) * 512],
                        start=True, stop=True)
                ex = expp.tile([128, 2, 512], F16, tag="ex",
                               name=f"ex{qt}_{c}_{kt}")
                nc.scalar.activation(ex[:, :, :], lg[:, :, :], AF.Exp,
                                     scale=0.125)
                dd = kt * 128 - qt * 512
                if dd >= 0:  # diagonal tile: multiplicative causal mask
                    for hh in range(2):
                        nc.vector.tensor_mul(ex[:, hh, :], ex[:, hh, :],
                                             mk[:, dd // 128, :])
                exs[(c, kt)] = ex

            def attn_v(c, kt):
                ex = exs.pop((c, kt))
                for hh in range(2):
                    h = 2 * c + hh
                    nc.tensor.matmul(
                        accs[(c, hh)][:],
                        lhsT=VP[:, kt, h * 66:h * 66 + 65],
                        rhs=ex[:, hh, :],
                        start=(kt == 0), stop=(kt == nkt - 1),
                        skip_group_check=True)

            for kt in range(nkt):
                for c in range(2):
                    lg_exp(c, kt)
                if kt >= 1:
                    for c in range(2):
                        attn_v(c, kt - 1)
            for c in range(2):
                attn_v(c, nkt - 1)

            # normalize: sv copy releases acc quickly; recip off critical path
            for c in range(2):
                for hh in range(2):
                    sv = smp.tile([65, 512], F32, tag="sv",
                                  name=f"sv{qt}_{c}_{hh}")
                    nc.vector.tensor_copy(sv[:], accs[(c, hh)][:])
                    rc = smp.tile([1, 512], F32, tag="rc",
                                  name=f"rc{qt}_{c}_{hh}")
                    nc.vector.reciprocal(rc[:], sv[64:65, :])
                    bc = smp.tile([64, 512], F32, tag="bc",
                                  name=f"bc{qt}_{c}_{hh}")
                    nc.gpsimd.partition_broadcast(bc[:], rc[:])
                    nc.vector.tensor_mul(
                        valsT[hh * 64:(hh + 1) * 64, c,
                              qt * 512:(qt + 1) * 512],
                        sv[0:64, :], bc[:])

            # O-projection for this q-tile's four 128-row chunks
            for tt in range(4 * qt, 4 * qt + 4):
                ot = opool.tile([128, E], F32, tag="ot", name=f"ot{tt}")
                for eo in range(2):
                    ps = psp.tile([128, 512], F32, tag="lg", bufs=2,
                                  name=f"ops{tt}_{eo}")
                    for c in range(2):
                        nc.tensor.matmul(
                            ps[:],
                            lhsT=valsT[:, c, tt * 128:(tt + 1) * 128],
                            rhs=wo[:, c, eo * 512:(eo + 1) * 512],
                            start=(c == 0), stop=(c == 1))
                    nc.vector.tensor_copy(ot[:, eo * 512:(eo + 1) * 512],
                                          ps[:])
                nc.sync.dma_start(out_d.ap()[tt * 128:(tt + 1) * 128, :],
                                  ot[:])

    nc.compile()
    return nc


def get_nc():
    if "nc" not in _CACHE:
        _CACHE["nc"] = _build()
    return _CACHE["nc"]


def _masks():
    i = np.arange(128)[:, None]
    j = np.arange(512)[None, :]
    m = np.zeros((4, 128, 512), dtype=NP16)
    for di in range(4):
        m[di] = (i + di * 128 <= j).astype(NP16)
    return m


def make_in_maps(query, key, value, Wq, bq, Wk, bk, Wv, bv, Wo, bo):
    query = np.asarray(query, np.float32)
    key = np.asarray(key, np.float32)
    value = np.asarray(value, np.float32)
    Wq, Wk, Wv, Wo = (np.asarray(a, np.float32) for a in (Wq, Wk, Wv, Wo))
    bq, bk, bv = (np.asarray(a, np.float32) for a in (bq, bk, bv))
    masks = _masks()
    vones = np.ones((128, 16, 4, 1), NP16)
    in_maps = []
    for c in range(NCORES):
        b, g = divmod(c, 4)
        sl = slice(g * EL, (g + 1) * EL)
        in_maps.append({
            "xqT": np.ascontiguousarray(query[b].T).astype(NP16),
            "xkT": np.ascontiguousarray(key[b].T).astype(NP16),
            "xvT": np.ascontiguousarray(value[b].T).astype(NP16),
            "wqT": np.ascontiguousarray(Wq[sl, :].T).astype(NP16),
            "wkT": np.ascontiguousarray(Wk[sl, :].T).astype(NP16),
            "wvT": np.ascontiguousarray(Wv[sl, :].T).astype(NP16),
            "woT": np.ascontiguousarray(Wo[:, sl].T).astype(NP16),
            "bq": np.ascontiguousarray(bq[sl]),
            "bk": np.ascontiguousarray(bk[sl]),
            "bv": np.ascontiguousarray(bv[sl]),
            "vones": vones,
            "masks": masks,
        })
    return in_maps


def run(inputs, trace=False, tmpdir=None):
    """Run on 8 cores; returns (full_output, BassKernelResults)."""
    nc = get_nc()
    in_maps = make_in_maps(**inputs)
    res = bass_utils.run_bass_kernel_spmd(
        nc, in_maps, list(range(NCORES)), trace=trace, tmpdir=tmpdir)
    bo = np.asarray(inputs["bo"], np.float32)
    out = np.zeros((B, S, E), np.float32)
    for c in range(NCORES):
        out[c // 4] += res.results[c]["out"]
    out += bo[None, None, :]
    return out, res


def kernel(**inputs):
    out, _ = run(inputs)
    return out


# revision 7
# speedup vs baseline: 1.2754x; 1.2754x over previous
"""Multi-head causal attention (B=2, S=2048, E=1024, H=16, D=64) on 8 TRN2
NeuronCores.

Sharding (data + tensor parallel, Megatron-style):
  core c -> batch b = c // 4, head group g = c % 4 (4 heads, e' = 256 cols).
  Wq/Wk/Wv column-sharded ([256, 1024] slices), Wo row-sharded
  ([1024, 256] slice); each core produces a partial output [2048, 1024]
  which the host sums per batch group (the Megatron all-reduce) and adds bo.

Per-core device kernel (matmul operands fp16, accumulate fp32 in PSUM):
  K^T = Wk_l x_k^T + bk  [256, 2048]   (e' on partitions -> heads x 64)
  Q^T = Wq_l x_q^T + bq  [256, 2048]
  V'  = [x_v Wv_l^T + bv | 1]  (ones col -> softmax denominator)
  attention in S^T orientation: per (q-tile 512, head-pair chunk), S^T
  tiles [128 k, 512 q] via PE with 2 heads packed in PE row groups
  0-63/64-127, exp on ACT (1/8 scale folded), multiplicative causal mask
  on the exp (diagonal tiles only), acc += V'^T @ P^T accumulated in PSUM
  [65, 512] whose row 64 is the softmax denominator; normalize via DVE
  reciprocal + GpSimd partition broadcast; O-projection from vals^T,
  emitted per q-tile. The k-loop interleaves both head-pair chunks and
  software-pipelines attnV one step behind exp so ACT stays saturated
  while PE works.
"""
import sys
import os

sys.path.insert(0, "/opt/trn_rl_repo")

import numpy as np
from contextlib import ExitStack

import concourse.bass as bass  # noqa: E402
import concourse.mybir as mybir  # noqa: E402
import concourse.tile as tile  # noqa: E402
from concourse import bacc, bass_utils  # noqa: E402

bass_utils.upload_artifacts = lambda d: f"local:{d}"

B, S, E, H, D = 2, 2048, 1024, 16, 64
NCORES = 8
EL = 256  # e' columns per core (4 heads)
F32 = mybir.dt.float32
F16 = mybir.dt.float16
AF = mybir.ActivationFunctionType
NP16 = np.float16

_CACHE = {}


def _build():
    nc = bacc.Bacc("TRN2", target_bir_lowering=False, debug=False)

    xq_d = nc.dram_tensor("xqT", [E, S], F16, kind="ExternalInput")
    xk_d = nc.dram_tensor("xkT", [E, S], F16, kind="ExternalInput")
    xv_d = nc.dram_tensor("xvT", [E, S], F16, kind="ExternalInput")
    wq_d = nc.dram_tensor("wqT", [E, EL], F16, kind="ExternalInput")
    wk_d = nc.dram_tensor("wkT", [E, EL], F16, kind="ExternalInput")
    wv_d = nc.dram_tensor("wvT", [E, EL], F16, kind="ExternalInput")
    wo_d = nc.dram_tensor("woT", [EL, E], F16, kind="ExternalInput")
    bq_d = nc.dram_tensor("bq", [EL], F32, kind="ExternalInput")
    bk_d = nc.dram_tensor("bk", [EL], F32, kind="ExternalInput")
    bv_d = nc.dram_tensor("bv", [EL], F32, kind="ExternalInput")
    vones_d = nc.dram_tensor("vones", [128, 16, 4, 1], F16, kind="ExternalInput")
    mask_d = nc.dram_tensor("masks", [4, 128, 512], F16, kind="ExternalInput")
    out_d = nc.dram_tensor("out", [S, E], F32, kind="ExternalOutput")

    with tile.TileContext(nc) as tc, ExitStack() as ctx:
        cpool = ctx.enter_context(tc.tile_pool(name="const", bufs=1))
        psp = ctx.enter_context(tc.tile_pool(name="psp", bufs=2, space="PSUM"))
        expp = ctx.enter_context(tc.tile_pool(name="expp", bufs=6))
        opool = ctx.enter_context(tc.tile_pool(name="op", bufs=4))
        smp = ctx.enter_context(tc.tile_pool(name="smp", bufs=4))

        # ---- constants + inputs, in consumption order (K, V, Q, then O) ----
        wk = cpool.tile([128, 8, EL], F16, tag="wk")
        nc.sync.dma_start(wk[:], wk_d.ap().rearrange("(k p) m -> p k m", p=128))
        bkt = cpool.tile([128, 2], F32, tag="bkt")
        nc.sync.dma_start(bkt[:], bk_d.ap().rearrange("(c p) -> p c", p=128))
        xk = cpool.tile([128, 8, S], F16, tag="xk")
        nc.sync.dma_start(xk[:], xk_d.ap().rearrange("(k p) m -> p k m", p=128))

        wv = cpool.tile([128, 8, EL], F16, tag="wv")
        nc.sync.dma_start(wv[:], wv_d.ap().rearrange("(k p) m -> p k m", p=128))
        bvr = cpool.tile([1, EL], F32, tag="bvr")
        nc.sync.dma_start(bvr[:], bv_d.ap().rearrange("(p m) -> p m", p=1))
        bvb = cpool.tile([128, EL], F32, tag="bvb")
        nc.gpsimd.partition_broadcast(bvb[:], bvr[:])
        xv = cpool.tile([128, 8, S], F16, tag="xv")
        nc.sync.dma_start(xv[:], xv_d.ap().rearrange("(k p) m -> p k m", p=128))

        wq = cpool.tile([128, 8, EL], F16, tag="wq")
        nc.sync.dma_start(wq[:], wq_d.ap().rearrange("(k p) m -> p k m", p=128))
        bqt = cpool.tile([128, 2], F32, tag="bqt")
        nc.sync.dma_start(bqt[:], bq_d.ap().rearrange("(c p) -> p c", p=128))
        xq = cpool.tile([128, 8, S], F16, tag="xq")
        nc.sync.dma_start(xq[:], xq_d.ap().rearrange("(k p) m -> p k m", p=128))

        mk = cpool.tile([128, 4, 512], F16, tag="mk")
        nc.sync.dma_start(mk[:], mask_d.ap().rearrange("k p m -> p k m"))
        wo = cpool.tile([128, 2, E], F16, tag="wo")
        nc.sync.dma_start(wo[:], wo_d.ap().rearrange("(c p) m -> p c m", p=128))

        KT = cpool.tile([128, 2, S], F16, tag="KT")
        QT = cpool.tile([128, 2, S], F16, tag="QT")
        VP = cpool.tile([128, 16, 4 * 66], F16, tag="VP")  # 66: 4B-aligned blocks
        valsT = cpool.tile([128, 2, S], F16, tag="valsT")

        # ones columns of V' (col 64 of each 66-block)
        nc.sync.dma_start(
            VP[:].rearrange("p k (h x) -> p k h x", h=4)[:, :, :, 64:65],
            vones_d.ap(),
        )

        # ---- transposed projections: out^T[e', t] = W x^T + b ----
        def proj_T(x_t, w_t, bias_t, out_t, pfx):
            for c in range(2):
                for tt in range(4):
                    ps = psp.tile([128, 512], F32, tag="lg", bufs=3,
                                  name=f"{pfx}ps{c}_{tt}")
                    for k in range(8):
                        nc.tensor.matmul(
                            ps[:],
                            lhsT=w_t[:, k, c * 128:(c + 1) * 128],
                            rhs=x_t[:, k, tt * 512:(tt + 1) * 512],
                            start=(k == 0), stop=(k == 7))
                    nc.vector.tensor_scalar_add(
                        out_t[:, c, tt * 512:(tt + 1) * 512],
                        ps[:], bias_t[:, c:c + 1])

        proj_T(xk, wk, bkt, KT, "k")

        # ---- V projection (natural layout) ----
        for t3 in range(16):
            ps = psp.tile([128, EL], F32, tag="lg", bufs=3, name=f"vps{t3}")
            for k in range(8):
                nc.tensor.matmul(
                    ps[:],
                    lhsT=xv[:, k, t3 * 128:(t3 + 1) * 128],
                    rhs=wv[:, k, :],
                    start=(k == 0), stop=(k == 7))
            nc.vector.tensor_add(
                VP[:, t3, :].rearrange("p (h x) -> p h x", h=4)[:, :, 0:64],
                ps[:].rearrange("p (h x) -> p h x", h=4),
                bvb[:].rearrange("p (h x) -> p h x", h=4))

        proj_T(xq, wq, bqt, QT, "q")

        # ---- attention + O-projection per q-tile ----
        for qt in range(4):
            nkt = 4 * qt + 4
            accs = {}
            for c in range(2):
                for hh in range(2):
                    accs[(c, hh)] = psp.tile([65, 512], F32, tag="acc",
                                             bufs=2, name=f"acc{qt}_{c}_{hh}")
            exs = {}

            def lg_exp(c, kt):
                lg = psp.tile([128, 2, 512], F32, tag="lg", bufs=3,
                              name=f"lg{qt}_{c}_{kt}")
                for hh in range(2):
                    nc.tensor.matmul(
                        lg[:, hh, :],
                        lhsT=KT[hh * 64:(hh + 1) * 64, c,
                                kt * 128:(kt + 1) * 128],
                        rhs=QT[hh * 64:(hh + 1) * 64, c,
                               qt * 512:(qt + 1) * 512],
                        start=True, stop=True)
                ex = expp.tile([128, 2, 512], F16, tag="ex",
                               name=f"ex{qt}_{c}_{kt}")
                nc.scalar.activation(ex[:, :, :], lg[:, :, :], AF.Exp,
                                     scale=0.125)
                dd = kt * 128 - qt * 512
                if dd >= 0:  # diagonal tile: multiplicative causal mask
                    for hh in range(2):
                        nc.vector.tensor_mul(ex[:, hh, :], ex[:, hh, :],
                                             mk[:, dd // 128, :])
                exs[(c, kt)] = ex

            def attn_v(c, kt):
                ex = exs.pop((c, kt))
                for hh in range(2):
                    h = 2 * c + hh
                    nc.tensor.matmul(
                        accs[(c, hh)][:],
                        lhsT=VP[:, kt, h * 66:h * 66 + 65],
                        rhs=ex[:, hh, :],
                        start=(kt == 0), stop=(kt == nkt - 1),
                        skip_group_check=True)

            for c in range(2):
                for kt in range(nkt):
                    lg_exp(c, kt)
                    if kt >= 2:
                        attn_v(c, kt - 2)
                attn_v(c, max(nkt - 2, 0))
                if nkt >= 2:
                    attn_v(c, nkt - 1)

            # normalize: sv copy releases acc; reciprocal on a DMA-transposed
            # [128, 4] column layout (4 elems/lane instead of 512)
            for c in range(2):
                for hh in range(2):
                    sv = smp.tile([65, 512], F32, tag="sv",
                                  name=f"sv{qt}_{c}_{hh}")
                    nc.vector.tensor_copy(sv[:], accs[(c, hh)][:])
                    lcol = smp.tile([128, 4], F32, tag="lcol",
                                    name=f"lcol{qt}_{c}_{hh}")
                    nc.sync.dma_start(
                        lcol[:, :],
                        sv[64:65, :].rearrange("p (a b) -> p a b", a=128))
                    rcol = smp.tile([128, 4], F32, tag="rcol",
                                    name=f"rcol{qt}_{c}_{hh}")
                    nc.vector.reciprocal(rcol[:, :], lcol[:, :])
                    rrow = smp.tile([1, 512], F32, tag="rrow",
                                    name=f"rrow{qt}_{c}_{hh}")
                    nc.sync.dma_start(
                        rrow[0:1, :].rearrange("p (a b) -> p a b", a=128),
                        rcol[:, :])
                    bc = smp.tile([64, 512], F32, tag="bc",
                                  name=f"bc{qt}_{c}_{hh}")
                    nc.gpsimd.partition_broadcast(bc[:], rrow[:])
                    nc.vector.tensor_mul(
                        valsT[hh * 64:(hh + 1) * 64, c,
                              qt * 512:(qt + 1) * 512],
                        sv[0:64, :], bc[:])

            # O-projection for this q-tile's four 128-row chunks
            for tt in range(4 * qt, 4 * qt + 4):
                ot = opool.tile([128, E], F32, tag="ot", name=f"ot{tt}")
                for eo in range(2):
                    ps = psp.tile([128, 512], F32, tag="lg", bufs=3,
                                  name=f"ops{tt}_{eo}")
                    for c in range(2):
                        nc.tensor.matmul(
                            ps[:],
                            lhsT=valsT[:, c, tt * 128:(tt + 1) * 128],
                            rhs=wo[:, c, eo * 512:(eo + 1) * 512],
                            start=(c == 0), stop=(c == 1))
                    nc.vector.tensor_copy(ot[:, eo * 512:(eo + 1) * 512],
                                          ps[:])
                nc.sync.dma_start(out_d.ap()[tt * 128:(tt + 1) * 128, :],
                                  ot[:])

    nc.compile()
    return nc


def get_nc():
    if "nc" not in _CACHE:
        _CACHE["nc"] = _build()
    return _CACHE["nc"]


def _masks():
    i = np.arange(128)[:, None]
    j = np.arange(512)[None, :]
    m = np.zeros((4, 128, 512), dtype=NP16)
    for di in range(4):
        m[di] = (i + di * 128 <= j).astype(NP16)
    return m


def make_in_maps(query, key, value, Wq, bq, Wk, bk, Wv, bv, Wo, bo):
    query = np.asarray(query, np.float32)
    key = np.asarray(key, np.float32)
    value = np.asarray(value, np.float32)
    Wq, Wk, Wv, Wo = (np.asarray(a, np.float32) for a in (Wq, Wk, Wv, Wo))
    bq, bk, bv = (np.asarray(a, np.float32) for a in (bq, bk, bv))
    masks = _masks()
    vones = np.ones((128, 16, 4, 1), NP16)
    in_maps = []
    for c in range(NCORES):
        b, g = divmod(c, 4)
        sl = slice(g * EL, (g + 1) * EL)
        in_maps.append({
            "xqT": np.ascontiguousarray(query[b].T).astype(NP16),
            "xkT": np.ascontiguousarray(key[b].T).astype(NP16),
            "xvT": np.ascontiguousarray(value[b].T).astype(NP16),
            "wqT": np.ascontiguousarray(Wq[sl, :].T).astype(NP16),
            "wkT": np.ascontiguousarray(Wk[sl, :].T).astype(NP16),
            "wvT": np.ascontiguousarray(Wv[sl, :].T).astype(NP16),
            "woT": np.ascontiguousarray(Wo[:, sl].T).astype(NP16),
            "bq": np.ascontiguousarray(bq[sl]),
            "bk": np.ascontiguousarray(bk[sl]),
            "bv": np.ascontiguousarray(bv[sl]),
            "vones": vones,
            "masks": masks,
        })
    return in_maps


def run(inputs, trace=False, tmpdir=None):
    """Run on 8 cores; returns (full_output, BassKernelResults)."""
    nc = get_nc()
    in_maps = make_in_maps(**inputs)
    res = bass_utils.run_bass_kernel_spmd(
        nc, in_maps, list(range(NCORES)), trace=trace, tmpdir=tmpdir)
    bo = np.asarray(inputs["bo"], np.float32)
    out = np.zeros((B, S, E), np.float32)
    for c in range(NCORES):
        out[c // 4] += res.results[c]["out"]
    out += bo[None, None, :]
    return out, res


def kernel(**inputs):
    out, _ = run(inputs)
    return out
